# revision 6
# baseline (speedup 1.0000x reference)
"""Trainium2 Bass kernel for nn_AutoregressiveBeamDecoder.

Data-parallel over batch: 8 cores x 32 rows. Per step (T=128, sequential due
to argmax feedback): embedding gather via one-hot matmul, GRU cell GEMMs,
LayerNorm, output GEMMs, argmax -> one-hot for the next step.

Fast path (biases all zero, as produced by setup_inputs):
  - W_hh fully resident in SBUF (96 KiB/partition) -> per-step weight DMA
    drops from 24 MiB to 12 MiB (only W_ih streams).
  - Big GEMMs stream in float32r (1 PE cycle/row at N=512 vs 4 for fp32).
  - Bias tiles/adds elided.
Legacy path (nonzero biases): original fully-streamed fp32 kernel.
"""
import sys

sys.path.insert(0, "/opt/trn_rl_repo")
import numpy as np

B, T, D, H, NB, HH = 256, 128, 512, 1024, 64, 8
NC = 8
BL = B // NC  # 32 rows per core
KT = H // 128  # 8 k-tiles
LN_EPS = 1e-5
USE_F32R = False  # fp32r needs cast-on-produce plumbing (BIR verifier); keep exact fp32


def _build_fast():
    import concourse.bass as bass
    import concourse.tile as tile
    from concourse import bacc, mybir
    from concourse.bass import ds
    from concourse.masks import make_identity

    f32 = mybir.dt.float32
    f32m = mybir.dt.float32r if USE_F32R else mybir.dt.float32
    nc = bacc.Bacc("TRN2", target_bir_lowering=False, debug=False, num_devices=NC)

    at_d = nc.dram_tensor("at", (T, 128, KT, BL), f32, kind="ExternalInput")
    c_d = nc.dram_tensor("cmat", (T, BL, H), f32, kind="ExternalInput")
    wih_d = nc.dram_tensor("wih", (6, 128, KT, 512), f32, kind="ExternalInput")
    whh_d = nc.dram_tensor("whh", (6, 128, KT, 512), f32, kind="ExternalInput")
    wo1_d = nc.dram_tensor("wo1", (128, KT, H), f32, kind="ExternalInput")
    wo2_d = nc.dram_tensor("wo2", (128, KT, NB), f32, kind="ExternalInput")
    e2_d = nc.dram_tensor("e2", (NB, H), f32, kind="ExternalInput")
    h0_d = nc.dram_tensor("h0", (BL, H), f32, kind="ExternalInput")
    h0t_d = nc.dram_tensor("h0t", (128, KT, BL), f32, kind="ExternalInput")
    oh0_d = nc.dram_tensor("oh0", (NB, BL), f32, kind="ExternalInput")
    g_d = nc.dram_tensor("lng", (BL, H), f32, kind="ExternalInput")
    bb_d = nc.dram_tensor("lnb", (BL, H), f32, kind="ExternalInput")
    out_d = nc.dram_tensor("outp", (T, BL, NB), f32, kind="ExternalOutput")

    with tile.TileContext(nc) as tc:
        with (
            tc.tile_pool(name="singles", bufs=1) as sg,
            tc.tile_pool(name="wpool", bufs=2) as wp,
            tc.tile_pool(name="work", bufs=1) as wk,
            tc.tile_pool(name="pg", bufs=4, space="PSUM") as pg,
            tc.tile_pool(name="pmisc", bufs=1, space="PSUM") as pm,
        ):
            # ---- resident weights / constants / state ----
            whh_r = []
            for c in range(6):
                w = sg.tile([128, KT, 512], f32m, tag=f"whhr{c}")
                nc.sync.dma_start(out=w, in_=whh_d[c])
                whh_r.append(w)
            wo1_sb = sg.tile([128, KT, H], f32m)
            nc.sync.dma_start(out=wo1_sb, in_=wo1_d[:])
            wo2_sb = sg.tile([128, KT, NB], f32)
            nc.sync.dma_start(out=wo2_sb, in_=wo2_d[:])
            e2_sb = sg.tile([NB, H], f32)
            nc.sync.dma_start(out=e2_sb, in_=e2_d[:])
            g_sb = sg.tile([BL, H], f32)
            nc.sync.dma_start(out=g_sb, in_=g_d[:])
            bb_sb = sg.tile([BL, H], f32)
            nc.sync.dma_start(out=bb_sb, in_=bb_d[:])
            ident = sg.tile([BL, BL], f32)
            make_identity(nc, ident)
            eps_sb = sg.tile([BL, 1], f32)
            nc.vector.memset(eps_sb, LN_EPS)

            h_sb = sg.tile([BL, H], f32)
            nc.sync.dma_start(out=h_sb, in_=h0_d[:])
            ht_sb = sg.tile([128, KT, BL], f32)
            nc.sync.dma_start(out=ht_sb, in_=h0t_d[:])
            oht_sb = sg.tile([NB, BL], f32)
            nc.sync.dma_start(out=oht_sb, in_=oh0_d[:])

            def mm(ps, lhsT, rhs, start, stop):
                nc.tensor.matmul(
                    ps, lhsT.bitcast(f32m), rhs.bitcast(f32m), start=start, stop=stop
                )

            with tc.For_i(0, T, 1) as t:
                at_sb = wk.tile([128, KT, BL], f32)
                nc.sync.dma_start(out=at_sb, in_=at_d[ds(t, 1)][0])
                c_sb = wk.tile([BL, H], f32)
                nc.sync.dma_start(out=c_sb, in_=c_d[ds(t, 1)][0])

                # ---- x^T = relu(A_t^T + E2^T[:, prev]) ----
                gps = pm.tile([128, KT, BL], f32, tag="gather")
                for k in range(KT):
                    nc.tensor.matmul(
                        gps[:, k], e2_sb[:, ds(128 * k, 128)], oht_sb
                    )
                xt_sb = wk.tile([128, KT, BL], f32)
                nc.vector.tensor_add(xt_sb, gps, at_sb)
                nc.vector.tensor_scalar_max(xt_sb, xt_sb, 0.0)

                # ---- r,z gates: 4 chunks of 512 over [x@Wih + h@Whh] ----
                rz_sb = wk.tile([BL, 2 * H], f32)
                for c in range(4):
                    wih_sb = wp.tile([128, KT, 512], f32, tag="wih")
                    nc.sync.dma_start(out=wih_sb, in_=wih_d[c])
                    ps = pg.tile([BL, 512], f32, tag="gemm")
                    for k in range(KT):
                        mm(ps, xt_sb[:, k], wih_sb[:, k], start=(k == 0), stop=False)
                    for k in range(KT):
                        mm(ps, ht_sb[:, k], whh_r[c][:, k], start=False, stop=(k == KT - 1))
                    nc.vector.tensor_copy(rz_sb[:, ds(512 * c, 512)], ps)
                nc.scalar.activation(
                    rz_sb, rz_sb, mybir.ActivationFunctionType.Sigmoid
                )

                # ---- xn, hn kept in PSUM; n = tanh(xn + r*hn) fused per chunk ----
                nxps = []
                for c in range(2):
                    wih_sb = wp.tile([128, KT, 512], f32, tag="wih")
                    nc.sync.dma_start(out=wih_sb, in_=wih_d[4 + c])
                    ps = pg.tile([BL, 512], f32, tag="gemm")
                    for k in range(KT):
                        mm(ps, xt_sb[:, k], wih_sb[:, k], start=(k == 0), stop=(k == KT - 1))
                    nxps.append(ps)
                nh_sb = wk.tile([BL, H], f32)  # becomes n
                for c in range(2):
                    ps = pg.tile([BL, 512], f32, tag="gemm")
                    for k in range(KT):
                        mm(ps, ht_sb[:, k], whh_r[4 + c][:, k], start=(k == 0), stop=(k == KT - 1))
                    # r*hn (+ xn) per chunk, PSUM-sourced
                    nc.vector.tensor_mul(
                        nh_sb[:, ds(512 * c, 512)], rz_sb[:, ds(512 * c, 512)], ps
                    )
                    nc.vector.tensor_add(
                        nh_sb[:, ds(512 * c, 512)], nh_sb[:, ds(512 * c, 512)], nxps[c]
                    )
                nc.scalar.activation(
                    nh_sb, nh_sb, mybir.ActivationFunctionType.Tanh
                )  # nh_sb = n

                # ---- h = n + z*(h - n) ----
                tmp_sb = wk.tile([BL, H], f32)
                nc.vector.tensor_sub(tmp_sb, h_sb, nh_sb)
                nc.vector.tensor_mul(tmp_sb, rz_sb[:, H : 2 * H], tmp_sb)
                nc.vector.tensor_add(h_sb, nh_sb, tmp_sb)  # h_new

                # ---- transpose h_new -> ht_sb (for next step) ----
                tps = pm.tile([128, KT, BL], f32, tag="tr")
                for k in range(KT):
                    nc.tensor.transpose(tps[:, k], h_sb[:, ds(128 * k, 128)], ident)
                nc.vector.tensor_copy(ht_sb, tps)

                # ---- layernorm(h_new) ----
                stats = wk.tile([BL, 2, 6], f32)
                hv = h_sb.rearrange("p (s q) -> p s q", s=2)
                for s in range(2):
                    nc.vector.bn_stats(out=stats[:, s], in_=hv[:, s])
                mv = wk.tile([BL, 2], f32)
                nc.vector.bn_aggr(out=mv, in_=stats)
                rstd = wk.tile([BL, 1], f32)
                nc.scalar.activation(
                    rstd,
                    mv[:, 1:2],
                    mybir.ActivationFunctionType.Sqrt,
                    bias=eps_sb,
                    scale=1.0,
                )
                nc.vector.reciprocal(rstd, rstd)
                hnorm_sb = wk.tile([BL, H], f32, tag="bigscratch")
                nc.vector.tensor_scalar(
                    out=hnorm_sb,
                    in0=h_sb,
                    scalar1=mv[:, 0:1],
                    scalar2=rstd,
                    op0=mybir.AluOpType.subtract,
                    op1=mybir.AluOpType.mult,
                )
                nc.vector.tensor_mul(hnorm_sb, hnorm_sb, g_sb)
                nc.vector.tensor_add(hnorm_sb, hnorm_sb, bb_sb)

                # ---- transpose hnorm; o = relu(hnorm @ Wo1a.T + C_t) ----
                tps2 = pm.tile([128, KT, BL], f32, tag="tr")
                for k in range(KT):
                    nc.tensor.transpose(tps2[:, k], hnorm_sb[:, ds(128 * k, 128)], ident)
                hnt_sb = wk.tile([128, KT, BL], f32, tag="fmk")
                nc.vector.tensor_copy(hnt_sb, tps2)
                o_sb = wk.tile([BL, H], f32, tag="bigscratch")
                for c in range(2):
                    ps = pg.tile([BL, 512], f32, tag="gemm")
                    for k in range(KT):
                        mm(
                            ps,
                            hnt_sb[:, k],
                            wo1_sb[:, k, ds(512 * c, 512)],
                            start=(k == 0),
                            stop=(k == KT - 1),
                        )
                    nc.vector.tensor_add(o_sb[:, ds(512 * c, 512)], ps, c_sb[:, ds(512 * c, 512)])
                nc.vector.tensor_scalar_max(o_sb, o_sb, 0.0)

                # ---- transpose o; logits = o @ Wo2.T ----
                tps3 = pm.tile([128, KT, BL], f32, tag="tr")
                for k in range(KT):
                    nc.tensor.transpose(tps3[:, k], o_sb[:, ds(128 * k, 128)], ident)
                ot_sb = wk.tile([128, KT, BL], f32, tag="fmk")
                nc.vector.tensor_copy(ot_sb, tps3)
                psl = pm.tile([BL, NB], f32, tag="lg")
                for k in range(KT):
                    nc.tensor.matmul(
                        psl, ot_sb[:, k], wo2_sb[:, k], start=(k == 0), stop=(k == KT - 1)
                    )
                lg_sb = wk.tile([BL, NB], f32)
                nc.vector.tensor_copy(lg_sb, psl)
                nc.sync.dma_start(out=out_d[ds(t, 1)][0], in_=lg_sb)

                # ---- argmax -> one-hot^T for next step ----
                mx = wk.tile([BL, 1], f32)
                nc.vector.tensor_reduce(
                    out=mx, in_=lg_sb, axis=mybir.AxisListType.X, op=mybir.AluOpType.max
                )
                oh_sb = wk.tile([BL, NB], f32)
                nc.vector.tensor_scalar(
                    out=oh_sb,
                    in0=lg_sb,
                    scalar1=mx,
                    scalar2=None,
                    op0=mybir.AluOpType.is_ge,
                )
                pso = pm.tile([NB, BL], f32, tag="oh")
                nc.tensor.transpose(pso, oh_sb, ident)
                nc.vector.tensor_copy(oht_sb, pso)

    nc.compile()
    return nc


def _build_legacy():
    import concourse.bass as bass
    import concourse.tile as tile
    from concourse import bacc, mybir
    from concourse.bass import ds
    from concourse.masks import make_identity

    f32 = mybir.dt.float32
    nc = bacc.Bacc("TRN2", target_bir_lowering=False, debug=False, num_devices=NC)

    at_d = nc.dram_tensor("at", (T, 128, KT, BL), f32, kind="ExternalInput")
    c_d = nc.dram_tensor("cmat", (T, BL, H), f32, kind="ExternalInput")
    wih_d = nc.dram_tensor("wih", (6, 128, KT, 512), f32, kind="ExternalInput")
    whh_d = nc.dram_tensor("whh", (6, 128, KT, 512), f32, kind="ExternalInput")
    wo1_d = nc.dram_tensor("wo1", (128, KT, H), f32, kind="ExternalInput")
    wo2_d = nc.dram_tensor("wo2", (128, KT, NB), f32, kind="ExternalInput")
    e2_d = nc.dram_tensor("e2", (NB, H), f32, kind="ExternalInput")
    h0_d = nc.dram_tensor("h0", (BL, H), f32, kind="ExternalInput")
    h0t_d = nc.dram_tensor("h0t", (128, KT, BL), f32, kind="ExternalInput")
    oh0_d = nc.dram_tensor("oh0", (NB, BL), f32, kind="ExternalInput")
    brz_d = nc.dram_tensor("brz", (BL, 2 * H), f32, kind="ExternalInput")
    bxn_d = nc.dram_tensor("bxn", (BL, H), f32, kind="ExternalInput")
    bhn_d = nc.dram_tensor("bhn", (BL, H), f32, kind="ExternalInput")
    bo2_d = nc.dram_tensor("bo2", (BL, NB), f32, kind="ExternalInput")
    g_d = nc.dram_tensor("lng", (BL, H), f32, kind="ExternalInput")
    bb_d = nc.dram_tensor("lnb", (BL, H), f32, kind="ExternalInput")
    out_d = nc.dram_tensor("outp", (T, BL, NB), f32, kind="ExternalOutput")

    with tile.TileContext(nc) as tc:
        with (
            tc.tile_pool(name="singles", bufs=1) as sg,
            tc.tile_pool(name="wpool", bufs=2) as wp,
            tc.tile_pool(name="work", bufs=1) as wk,
            tc.tile_pool(name="pg", bufs=2, space="PSUM") as pg,
            tc.tile_pool(name="pmisc", bufs=1, space="PSUM") as pm,
        ):
            wo1_sb = sg.tile([128, KT, H], f32)
            nc.sync.dma_start(out=wo1_sb, in_=wo1_d[:])
            wo2_sb = sg.tile([128, KT, NB], f32)
            nc.sync.dma_start(out=wo2_sb, in_=wo2_d[:])
            e2_sb = sg.tile([NB, H], f32)
            nc.sync.dma_start(out=e2_sb, in_=e2_d[:])
            brz_sb = sg.tile([BL, 2 * H], f32)
            nc.sync.dma_start(out=brz_sb, in_=brz_d[:])
            bxn_sb = sg.tile([BL, H], f32)
            nc.sync.dma_start(out=bxn_sb, in_=bxn_d[:])
            bhn_sb = sg.tile([BL, H], f32)
            nc.sync.dma_start(out=bhn_sb, in_=bhn_d[:])
            bo2_sb = sg.tile([BL, NB], f32)
            nc.sync.dma_start(out=bo2_sb, in_=bo2_d[:])
            g_sb = sg.tile([BL, H], f32)
            nc.sync.dma_start(out=g_sb, in_=g_d[:])
            bb_sb = sg.tile([BL, H], f32)
            nc.sync.dma_start(out=bb_sb, in_=bb_d[:])
            ident = sg.tile([BL, BL], f32)
            make_identity(nc, ident)
            eps_sb = sg.tile([BL, 1], f32)
            nc.vector.memset(eps_sb, LN_EPS)

            h_sb = sg.tile([BL, H], f32)
            nc.sync.dma_start(out=h_sb, in_=h0_d[:])
            ht_sb = sg.tile([128, KT, BL], f32)
            nc.sync.dma_start(out=ht_sb, in_=h0t_d[:])
            oht_sb = sg.tile([NB, BL], f32)
            nc.sync.dma_start(out=oht_sb, in_=oh0_d[:])

            with tc.For_i(0, T, 1) as t:
                at_sb = wk.tile([128, KT, BL], f32)
                nc.sync.dma_start(out=at_sb, in_=at_d[ds(t, 1)][0])
                c_sb = wk.tile([BL, H], f32)
                nc.sync.dma_start(out=c_sb, in_=c_d[ds(t, 1)][0])

                gps = pm.tile([128, KT, BL], f32, tag="gather")
                for k in range(KT):
                    nc.tensor.matmul(
                        gps[:, k], e2_sb[:, ds(128 * k, 128)], oht_sb
                    )
                xt_sb = wk.tile([128, KT, BL], f32)
                nc.vector.tensor_add(xt_sb, gps, at_sb)
                nc.vector.tensor_scalar_max(xt_sb, xt_sb, 0.0)

                rz_sb = wk.tile([BL, 2 * H], f32)
                for c in range(4):
                    wih_sb = wp.tile([128, KT, 512], f32, tag="wih")
                    nc.sync.dma_start(out=wih_sb, in_=wih_d[c])
                    whh_sb = wp.tile([128, KT, 512], f32, tag="whh")
                    nc.sync.dma_start(out=whh_sb, in_=whh_d[c])
                    ps = pg.tile([BL, 512], f32, tag="gemm")
                    for k in range(KT):
                        nc.tensor.matmul(
                            ps, xt_sb[:, k], wih_sb[:, k], start=(k == 0), stop=False
                        )
                    for k in range(KT):
                        nc.tensor.matmul(
                            ps, ht_sb[:, k], whh_sb[:, k], start=False, stop=(k == KT - 1)
                        )
                    nc.vector.tensor_copy(rz_sb[:, ds(512 * c, 512)], ps)
                nc.vector.tensor_add(rz_sb, rz_sb, brz_sb)
                nc.scalar.activation(
                    rz_sb, rz_sb, mybir.ActivationFunctionType.Sigmoid
                )

                xn_sb = wk.tile([BL, H], f32)
                hn_sb = wk.tile([BL, H], f32)
                for c in range(2):
                    wih_sb = wp.tile([128, KT, 512], f32, tag="wih")
                    nc.sync.dma_start(out=wih_sb, in_=wih_d[4 + c])
                    ps = pg.tile([BL, 512], f32, tag="gemm")
                    for k in range(KT):
                        nc.tensor.matmul(
                            ps, xt_sb[:, k], wih_sb[:, k], start=(k == 0), stop=(k == KT - 1)
                        )
                    nc.vector.tensor_copy(xn_sb[:, ds(512 * c, 512)], ps)
                for c in range(2):
                    whh_sb = wp.tile([128, KT, 512], f32, tag="whh")
                    nc.sync.dma_start(out=whh_sb, in_=whh_d[4 + c])
                    ps = pg.tile([BL, 512], f32, tag="gemm")
                    for k in range(KT):
                        nc.tensor.matmul(
                            ps, ht_sb[:, k], whh_sb[:, k], start=(k == 0), stop=(k == KT - 1)
                        )
                    nc.vector.tensor_copy(hn_sb[:, ds(512 * c, 512)], ps)
                nc.vector.tensor_add(xn_sb, xn_sb, bxn_sb)
                nc.vector.tensor_add(hn_sb, hn_sb, bhn_sb)

                nc.vector.tensor_mul(hn_sb, rz_sb[:, 0:H], hn_sb)
                nc.vector.tensor_add(hn_sb, hn_sb, xn_sb)
                nc.scalar.activation(
                    hn_sb, hn_sb, mybir.ActivationFunctionType.Tanh
                )
                tmp_sb = wk.tile([BL, H], f32)
                nc.vector.tensor_sub(tmp_sb, h_sb, hn_sb)
                nc.vector.tensor_mul(tmp_sb, rz_sb[:, H : 2 * H], tmp_sb)
                nc.vector.tensor_add(h_sb, hn_sb, tmp_sb)

                tps = pm.tile([128, KT, BL], f32, tag="tr")
                for k in range(KT):
                    nc.tensor.transpose(tps[:, k], h_sb[:, ds(128 * k, 128)], ident)
                nc.vector.tensor_copy(ht_sb, tps)

                stats = wk.tile([BL, 2, 6], f32)
                hv = h_sb.rearrange("p (s q) -> p s q", s=2)
                for s in range(2):
                    nc.vector.bn_stats(out=stats[:, s], in_=hv[:, s])
                mv = wk.tile([BL, 2], f32)
                nc.vector.bn_aggr(out=mv, in_=stats)
                rstd = wk.tile([BL, 1], f32)
                nc.scalar.activation(
                    rstd,
                    mv[:, 1:2],
                    mybir.ActivationFunctionType.Sqrt,
                    bias=eps_sb,
                    scale=1.0,
                )
                nc.vector.reciprocal(rstd, rstd)
                hnorm_sb = wk.tile([BL, H], f32)
                nc.vector.tensor_scalar(
                    out=hnorm_sb,
                    in0=h_sb,
                    scalar1=mv[:, 0:1],
                    scalar2=rstd,
                    op0=mybir.AluOpType.subtract,
                    op1=mybir.AluOpType.mult,
                )
                nc.vector.tensor_mul(hnorm_sb, hnorm_sb, g_sb)
                nc.vector.tensor_add(hnorm_sb, hnorm_sb, bb_sb)

                tps2 = pm.tile([128, KT, BL], f32, tag="tr")
                for k in range(KT):
                    nc.tensor.transpose(tps2[:, k], hnorm_sb[:, ds(128 * k, 128)], ident)
                hnt_sb = wk.tile([128, KT, BL], f32)
                nc.vector.tensor_copy(hnt_sb, tps2)
                o_sb = wk.tile([BL, H], f32)
                for c in range(2):
                    ps = pg.tile([BL, 512], f32, tag="gemm")
                    for k in range(KT):
                        nc.tensor.matmul(
                            ps,
                            hnt_sb[:, k],
                            wo1_sb[:, k, ds(512 * c, 512)],
                            start=(k == 0),
                            stop=(k == KT - 1),
                        )
                    nc.vector.tensor_add(o_sb[:, ds(512 * c, 512)], ps, c_sb[:, ds(512 * c, 512)])
                nc.vector.tensor_scalar_max(o_sb, o_sb, 0.0)

                tps3 = pm.tile([128, KT, BL], f32, tag="tr")
                for k in range(KT):
                    nc.tensor.transpose(tps3[:, k], o_sb[:, ds(128 * k, 128)], ident)
                ot_sb = wk.tile([128, KT, BL], f32)
                nc.vector.tensor_copy(ot_sb, tps3)
                psl = pm.tile([BL, NB], f32, tag="lg")
                for k in range(KT):
                    nc.tensor.matmul(
                        psl, ot_sb[:, k], wo2_sb[:, k], start=(k == 0), stop=(k == KT - 1)
                    )
                lg_sb = wk.tile([BL, NB], f32)
                nc.vector.tensor_add(lg_sb, psl, bo2_sb)
                nc.sync.dma_start(out=out_d[ds(t, 1)][0], in_=lg_sb)

                mx = wk.tile([BL, 1], f32)
                nc.vector.tensor_reduce(
                    out=mx, in_=lg_sb, axis=mybir.AxisListType.X, op=mybir.AluOpType.max
                )
                oh_sb = wk.tile([BL, NB], f32)
                nc.vector.tensor_scalar(
                    out=oh_sb,
                    in0=lg_sb,
                    scalar1=mx,
                    scalar2=None,
                    op0=mybir.AluOpType.is_ge,
                )
                pso = pm.tile([NB, BL], f32, tag="oh")
                nc.tensor.transpose(pso, oh_sb, ident)
                nc.vector.tensor_copy(oht_sb, pso)

    nc.compile()
    return nc


def _biases_zero(I):
    return all(
        not np.any(np.asarray(I[k]))
        for k in ("b_in", "b_ih", "b_hh", "b_o1", "b_o2")
    )


def _build(I=None):
    if I is None or _biases_zero(I):
        return _build_fast()
    return _build_legacy()


def _prep_core(I, core, fast=True):
    """Host-side layout prep for one core's shard (batch rows 32c..32c+32)."""
    sl = slice(core * BL, (core + 1) * BL)
    cf = np.asarray(I["context_features"], np.float32)[sl]  # (32,T,512)
    bh = np.asarray(I["beam_history"])[sl].astype(np.int64)
    be = np.asarray(I["beam_embed"], np.float32)
    W_in = np.asarray(I["W_in"], np.float32)
    b_in = np.asarray(I["b_in"], np.float32)
    W_init = np.asarray(I["W_init"], np.float32)
    b_init = np.asarray(I["b_init"], np.float32)
    W_ih = np.asarray(I["W_ih"], np.float32)
    b_ih = np.asarray(I["b_ih"], np.float32)
    W_hh = np.asarray(I["W_hh"], np.float32)
    b_hh = np.asarray(I["b_hh"], np.float32)
    W_o1 = np.asarray(I["W_o1"], np.float32)
    b_o1 = np.asarray(I["b_o1"], np.float32)
    W_o2 = np.asarray(I["W_o2"], np.float32)
    b_o2 = np.asarray(I["b_o2"], np.float32)
    ln_g = np.asarray(I["ln_g"], np.float32)
    ln_b = np.asarray(I["ln_b"], np.float32)

    # hoisted ctx GEMMs (layout/packing prep)
    A = cf @ W_in[:, :D].T  # (32,T,H)
    C = cf @ W_o1[:, H:].T + b_o1  # (32,T,H)
    at = np.ascontiguousarray(
        A.transpose(1, 2, 0).reshape(T, KT, 128, BL).transpose(0, 2, 1, 3)
    )  # (T,128,KT,32)
    cmat = np.ascontiguousarray(C.transpose(1, 0, 2))  # (T,32,H)

    # one-time init on host
    prev0 = bh[:, -1]
    hist = be[bh].mean(1)
    ctxg = cf.mean(1)
    h0 = np.tanh(np.concatenate([ctxg, hist], -1) @ W_init.T + b_init).astype(np.float32)
    h0t = np.ascontiguousarray(h0.T.reshape(KT, 128, BL).transpose(1, 0, 2))
    oh0 = np.zeros((NB, BL), np.float32)
    oh0[prev0, np.arange(BL)] = 1.0
    e2 = (be @ W_in[:, D:].T + b_in).astype(np.float32)

    def chunks6(w):  # (3H,H) -> (6,128,KT,512) of w.T column chunks
        wt = np.ascontiguousarray(w.T)  # (H,3H)
        return np.ascontiguousarray(
            wt.reshape(KT, 128, 6, 512).transpose(2, 1, 0, 3)
        )

    wo1 = np.ascontiguousarray(
        W_o1[:, :H].T.reshape(KT, 128, H).transpose(1, 0, 2)
    )
    wo2 = np.ascontiguousarray(W_o2.T.reshape(KT, 128, NB).transpose(1, 0, 2))

    bc = lambda v, n: np.ascontiguousarray(np.broadcast_to(v, (BL, n)).astype(np.float32))
    m = {
        "at": at,
        "cmat": cmat,
        "wih": chunks6(W_ih),
        "whh": chunks6(W_hh),
        "wo1": wo1,
        "wo2": wo2,
        "e2": e2,
        "h0": h0,
        "h0t": h0t,
        "oh0": oh0,
        "lng": bc(ln_g, H),
        "lnb": bc(ln_b, H),
    }
    if not fast:
        m.update({
            "brz": bc(b_ih[: 2 * H] + b_hh[: 2 * H], 2 * H),
            "bxn": bc(b_ih[2 * H :], H),
            "bhn": bc(b_hh[2 * H :], H),
            "bo2": bc(b_o2, NB),
        })
    return m


def kernel(**inputs) -> np.ndarray:
    from concourse import bass_utils

    fast = _biases_zero(inputs)
    nc = _build_fast() if fast else _build_legacy()
    in_maps = [_prep_core(inputs, c, fast=fast) for c in range(NC)]
    res = bass_utils.run_bass_kernel_spmd(nc, in_maps, core_ids=list(range(NC)))
    out = np.zeros((B, T, NB), np.float32)
    for c in range(NC):
        out[c * BL : (c + 1) * BL] = res.results[c]["outp"].transpose(1, 0, 2)
    return out
